# revision 4
# baseline (speedup 1.0000x reference)
"""GRPO loss kernel for Trainium2 (8 NeuronCores, data-parallel over B*L rows).

Heavy part: per-row logsumexp over the vocab dim of logits (2, 1025, 50257) f32.
Rows (B*L = 2048) are sharded 256/core; each core streams its (256, 50257) slab
through SBUF and computes per-row sum(exp(x)) with fused ACT exp+accumulate.
Host finishes with log(), the token-logit gather, and the tiny (B, L) epilogue.
"""

import numpy as np

import concourse.bacc as bacc
import concourse.tile as tile
from concourse import mybir
from concourse.bass_utils import run_bass_kernel_spmd

B = 2
L = 1024
V = 50257
TEMPERATURE = 1.0
BETA = 0.04
EPS_LOW = 0.2
EPS_HIGH = 0.2

N_CORES = 8
ROWS_PER_CORE = (B * L) // N_CORES  # 256
P = 128                             # SBUF partitions
PT_TILES = ROWS_PER_CORE // P       # 2
FREE = 8192                         # free-dim tile -> 4 MiB DMAs
N_FT = (V + FREE - 1) // FREE       # 7 (6 full + 1105 tail)

_cache = {}


def _build_nc():
    # Bacc (not raw Bass): its compile() pass splits multi-sem waits into
    # EventSemaphore instructions — TRN2 allows only 1 wait per instruction.
    nc = bacc.Bacc("TRN2", target_bir_lowering=False)
    x = nc.dram_tensor("x", [ROWS_PER_CORE, V], mybir.dt.float32,
                       kind="ExternalInput")
    out = nc.dram_tensor("partials", [PT_TILES, P, N_FT], mybir.dt.float32,
                         kind="ExternalOutput")

    with tile.TileContext(nc) as tc:
        with (
            tc.tile_pool(name="xtiles", bufs=4) as xpool,
            tc.tile_pool(name="stats", bufs=2) as spool,
        ):
            for pt in range(PT_TILES):
                partials = spool.tile([P, N_FT], mybir.dt.float32)
                for ft in range(N_FT):
                    f0 = ft * FREE
                    w = min(V - f0, FREE)
                    xt = xpool.tile([P, FREE], mybir.dt.float32)
                    nc.sync.dma_start(
                        out=xt[:, :w],
                        in_=x[pt * P:(pt + 1) * P, f0:f0 + w],
                    )
                    nc.scalar.activation(
                        out=xt[:, :w],
                        in_=xt[:, :w],
                        func=mybir.ActivationFunctionType.Exp,
                        accum_out=partials[:, ft:ft + 1],
                    )
                nc.sync.dma_start(out=out[pt], in_=partials)
    nc.finalize()
    return nc


def _get_nc():
    if "nc" not in _cache:
        _cache["nc"] = _build_nc()
    return _cache["nc"]


def _run_device(logits, trace=False):
    """Returns per-row sum(exp(logit)) of shape (B*L,), plus the raw result."""
    rows = np.ascontiguousarray(
        logits[:, :L, :].reshape(B * L, V).astype(np.float32, copy=False))
    in_maps = [
        {"x": np.ascontiguousarray(rows[i * ROWS_PER_CORE:(i + 1) * ROWS_PER_CORE])}
        for i in range(N_CORES)
    ]
    res = run_bass_kernel_spmd(_get_nc(), in_maps,
                               core_ids=list(range(N_CORES)), trace=trace)
    part = np.stack([r["partials"] for r in res.results])   # (8, 2, 128, N_FT)
    sumexp = part.astype(np.float64).sum(axis=-1).reshape(B * L)
    return sumexp, res


def kernel(logits, completion_ids, advantages, old_logp, ref_logp,
           completion_mask, _trace=False, _want_res=False):
    sumexp, res = _run_device(logits, trace=_trace)

    lse = np.log(sumexp).reshape(B, L).astype(np.float32)        # (B, L)
    tok_logit = np.take_along_axis(
        logits[:, :L, :], completion_ids[..., None].astype(np.int64), axis=2
    )[..., 0].astype(np.float32)
    if TEMPERATURE != 1.0:
        tok_logit = tok_logit / np.float32(TEMPERATURE)
    logp = tok_logit - lse                                       # (B, L)

    coef_1 = np.exp(logp - old_logp)
    adv = advantages[:, None].astype(np.float32)                 # (B, 1)
    coef_2 = np.clip(coef_1, 1.0 - EPS_LOW, 1.0 + EPS_HIGH)
    loss1 = coef_1 * adv
    loss2 = coef_2 * adv
    per_token_loss = -np.minimum(loss1, loss2)

    diff = ref_logp.astype(np.float32) - logp
    kl = np.exp(diff) - diff - 1.0
    per_token_loss = per_token_loss + np.float32(BETA) * kl

    mask = completion_mask.astype(np.float32)
    mask_sum = max(mask.sum(), 1.0)
    kl_mean = (kl * mask).sum() / mask_sum
    is_clipped = (((coef_1 < 1.0 - EPS_LOW) & (adv < 0))
                  | ((coef_1 > 1.0 + EPS_HIGH) & (adv > 0)))
    clip_ratio = (is_clipped.astype(np.float32) * mask).sum() / mask_sum

    seq_lens = np.maximum(mask.sum(-1), 1.0)                     # (B,)
    reduced_loss = ((per_token_loss * mask).sum(-1) / seq_lens).mean()

    out = (np.float32(reduced_loss), np.float32(kl_mean), np.float32(clip_ratio))
    if _want_res:
        return out, res
    return out


# revision 7
# speedup vs baseline: 1.1980x; 1.1980x over previous
"""GRPO loss kernel for Trainium2 (8 NeuronCores, data-parallel over B*L rows).

Heavy part: per-row logsumexp over the vocab dim of logits (2, 1025, 50257) f32.
Rows (B*L = 2048) are sharded 256/core; each core streams its (256, 50257) slab
through SBUF and computes per-row sum(exp(x)) with fused ACT exp+accumulate.
Host finishes with log(), the token-logit gather, and the tiny (B, L) epilogue.
"""

import numpy as np

import concourse.bacc as bacc
import concourse.tile as tile
from concourse import mybir
from concourse.bass_utils import run_bass_kernel_spmd

B = 2
L = 1024
V = 50257
TEMPERATURE = 1.0
BETA = 0.04
EPS_LOW = 0.2
EPS_HIGH = 0.2

N_CORES = 8
ROWS_PER_CORE = (B * L) // N_CORES  # 256
P = 128                             # SBUF partitions
PT_TILES = ROWS_PER_CORE // P       # 2
FREE = 2048                         # free-dim tile -> 1 MiB DMAs
N_FT = (V + FREE - 1) // FREE       # 25 (24 full + 1105 tail)
BUFS = 24                           # deep pipeline; 24 x 8KB = 192KB/partition

_cache = {}


def _build_nc():
    # Bacc (not raw Bass): its compile() pass splits multi-sem waits into
    # EventSemaphore instructions — TRN2 allows only 1 wait per instruction.
    nc = bacc.Bacc("TRN2", target_bir_lowering=False)
    x = nc.dram_tensor("x", [ROWS_PER_CORE, V], mybir.dt.float32,
                       kind="ExternalInput")
    out = nc.dram_tensor("partials", [ROWS_PER_CORE, N_FT], mybir.dt.float32,
                         kind="ExternalOutput")

    with tile.TileContext(nc) as tc:
        with (
            tc.tile_pool(name="xtiles", bufs=BUFS) as xpool,
            tc.tile_pool(name="stats", bufs=2) as spool,
        ):
            for pt in range(PT_TILES):
                partials = spool.tile([P, N_FT], mybir.dt.float32)
                for ft in range(N_FT):
                    f0 = ft * FREE
                    w = min(V - f0, FREE)
                    xt = xpool.tile([P, FREE], mybir.dt.float32)
                    nc.sync.dma_start(
                        out=xt[:, :w],
                        in_=x[pt * P:(pt + 1) * P, f0:f0 + w],
                    )
                    nc.scalar.activation(
                        out=xt[:, :w],
                        in_=xt[:, :w],
                        func=mybir.ActivationFunctionType.Exp,
                        accum_out=partials[:, ft:ft + 1],
                    )
                nc.sync.dma_start(out=out[pt * P:(pt + 1) * P], in_=partials)
    nc.finalize()
    return nc


def _get_nc():
    if "nc" not in _cache:
        _cache["nc"] = _build_nc()
    return _cache["nc"]


def _run_device(logits, trace=False):
    """Returns per-row sum(exp(logit)) of shape (B*L,), plus the raw result."""
    rows = np.ascontiguousarray(
        logits[:, :L, :].reshape(B * L, V).astype(np.float32, copy=False))
    in_maps = [
        {"x": np.ascontiguousarray(rows[i * ROWS_PER_CORE:(i + 1) * ROWS_PER_CORE])}
        for i in range(N_CORES)
    ]
    res = run_bass_kernel_spmd(_get_nc(), in_maps,
                               core_ids=list(range(N_CORES)), trace=trace)
    part = np.stack([r["partials"] for r in res.results])   # (8, 256, N_FT)
    sumexp = part.astype(np.float64).sum(axis=-1).reshape(B * L)
    return sumexp, res


def kernel(logits, completion_ids, advantages, old_logp, ref_logp,
           completion_mask, _trace=False, _want_res=False):
    sumexp, res = _run_device(logits, trace=_trace)

    lse = np.log(sumexp).reshape(B, L).astype(np.float32)        # (B, L)
    tok_logit = np.take_along_axis(
        logits[:, :L, :], completion_ids[..., None].astype(np.int64), axis=2
    )[..., 0].astype(np.float32)
    if TEMPERATURE != 1.0:
        tok_logit = tok_logit / np.float32(TEMPERATURE)
    logp = tok_logit - lse                                       # (B, L)

    coef_1 = np.exp(logp - old_logp)
    adv = advantages[:, None].astype(np.float32)                 # (B, 1)
    coef_2 = np.clip(coef_1, 1.0 - EPS_LOW, 1.0 + EPS_HIGH)
    loss1 = coef_1 * adv
    loss2 = coef_2 * adv
    per_token_loss = -np.minimum(loss1, loss2)

    diff = ref_logp.astype(np.float32) - logp
    kl = np.exp(diff) - diff - 1.0
    per_token_loss = per_token_loss + np.float32(BETA) * kl

    mask = completion_mask.astype(np.float32)
    mask_sum = max(mask.sum(), 1.0)
    kl_mean = (kl * mask).sum() / mask_sum
    is_clipped = (((coef_1 < 1.0 - EPS_LOW) & (adv < 0))
                  | ((coef_1 > 1.0 + EPS_HIGH) & (adv > 0)))
    clip_ratio = (is_clipped.astype(np.float32) * mask).sum() / mask_sum

    seq_lens = np.maximum(mask.sum(-1), 1.0)                     # (B,)
    reduced_loss = ((per_token_loss * mask).sum(-1) / seq_lens).mean()

    out = (np.float32(reduced_loss), np.float32(kl_mean), np.float32(clip_ratio))
    if _want_res:
        return out, res
    return out
